# revision 53
# baseline (speedup 1.0000x reference)
"""GridPoolingLayer kernel for Trainium2 (8 NeuronCores, Bass/Tile).

Semantics: the 1D binary masks partition H/W into maximal runs of constant
value; the layer replaces every grid cell with its mean (keep_size=True).
The op is separable; per core (channels sharded 8 ways, 32 ch/core), fp16:

  A+B+D) ONE PE stage per 128-row output window:
       psum[r, wseg] = sum_l sum_h F[h, r] * x[h, (wseg, l)]
     F[h, r] = 1/L_h iff rows h and r share a row segment, so the matmul
     contraction over h does the row pool AND the one-hot row expansion,
     while PSUM accumulation over the within-segment offset l does the
     col pool.  x is stored l-major per column-length class (host
     permutation) so every moving operand is contiguous.  x stays
     resident in SBUF (128 KB/partition of the 208 KB budget).
  C) col expand     ot[r, w] = psum[r, seg(w)] / L_w -- step-0 broadcast
     APs, one instruction per (length class x psum chunk) piece, split
     between Vector and Scalar (GpSimd cannot read PSUM).  Two clean 2D
     stores per window (a DMA broadcast source would be pinned to single
     SBUF ports, hence the on-chip expansion).

The host un-permutes the column classes (pure gather) while unsharding
and upcasts fp16 -> fp32.  fp16 keeps HBM traffic at 16 MB in + 16 MB
out per core (vs 64 MB for fp32) and runs the PE at full 16-bit rate;
the 2e-2 harness tolerance leaves ~40x margin over fp16 rounding noise
(and the col sums accumulate in fp32 PSUM).
"""

import math
import numpy as np

H, W, C = 512, 512, 256
NCORES = 8
CS = C // NCORES  # 32 channels per core
P = 128
FW = W * CS       # row free size in elements (16384)
PSW = 512         # psum bank width in fp32 elems
CHUNK2 = 1024     # psum tile width (fp32 elems, two banks)
LOAD_SPLIT = 4    # DMAs per h-chunk of x (pipelining granularity)


def _segments(mask):
    m = np.asarray(mask).ravel()
    change = np.nonzero(m[1:] != m[:-1])[0] + 1
    bounds = np.concatenate([[0], change, [len(m)]]).astype(np.int64)
    return [(int(bounds[i]), int(bounds[i + 1])) for i in range(len(bounds) - 1)]


def _plan(row_segs, col_segs):
    """Host-side geometry planning shared by program build + data prep."""
    from collections import defaultdict

    S_h, S_w = len(row_segs), len(col_segs)
    Kh = math.ceil(H / P)

    # ---- column side: class-sorted device order -------------------------
    by_len = defaultdict(list)
    for t, (u, v) in enumerate(col_segs):
        by_len[v - u].append(t)
    perm_cols = [t for L in sorted(by_len) for t in by_len[L]]

    wperm = np.empty(W, dtype=np.int64)   # dev w unit -> orig w
    off = 0
    slot_bases = []  # (L, n, slot0) per class, slot-ordered
    sl = 0
    for L in sorted(by_len):
        ts = by_len[L]
        for t in ts:
            u, v = col_segs[t]
            wperm[off:off + L] = np.arange(u, v)
            off += L
        slot_bases.append((L, len(ts), sl))
        sl += len(ts)
    assert off == W

    # wstart[slot] = dev w unit where that (class-sorted) segment starts
    wstart = np.empty(S_w + 1, dtype=np.int64)
    pos = 0
    for (L, n, slot0) in slot_bases:
        for j in range(n):
            wstart[slot0 + j] = pos
            pos += L
    wstart[S_w] = pos

    # x element layout: class blocks in slot order, each [L, n, CS]
    # l-major; cls_x0[class] = elem offset of the block
    cls_x0 = []
    x0 = 0
    for (L, n, slot0) in slot_bases:
        cls_x0.append(x0)
        x0 += L * n * CS
    assert x0 == FW

    # ---- matmul piece groups, keyed by psum half (512 fp32 = 16 slots) --
    # group: (ps_off, width, [x_off per l]) -- one PSUM-accumulation set
    mm_groups = defaultdict(list)
    for ci_, (L, n, slot0) in enumerate(slot_bases):
        i = 0
        while i < n:
            hi = (slot0 + i) * CS // PSW
            room = ((hi + 1) * PSW - (slot0 + i) * CS) // CS
            take = min(room, n - i)
            x_offs = [
                cls_x0[ci_] + l * n * CS + i * CS for l in range(L)
            ]
            mm_groups[hi].append(
                ((slot0 + i) * CS, take * CS, x_offs)
            )
            i += take

    # ---- expansion pieces, keyed by psum chunk (CHUNK2) -----------------
    slots_per_chunk = CHUNK2 // CS
    exp_pieces = defaultdict(list)
    for (L, n, slot0) in slot_bases:
        i = 0
        while i < n:
            ch = (slot0 + i) // slots_per_chunk
            room = (ch + 1) * slots_per_chunk - (slot0 + i)
            take = min(room, n - i)
            exp_pieces[ch].append(
                (L, take, slot0 + i, int(wstart[slot0 + i]))
            )
            i += take

    n_chunks = math.ceil(S_w * CS / CHUNK2)
    # last output column (element) finalized once chunk ci is expanded
    chunk_out_end = [
        int(wstart[min((ci + 1) * slots_per_chunk, S_w)]) * CS
        for ci in range(n_chunks)
    ]

    # ---- output windows: 128 consecutive rows, with the h-chunks their --
    # segments straddle
    seg_of_h = np.empty(H, dtype=np.int64)
    for s, (va, vb) in enumerate(row_segs):
        seg_of_h[va:vb] = s
    windows = []
    for a in range(0, H, P):
        wlen = min(P, H - a)
        h_lo = row_segs[seg_of_h[a]][0]
        h_hi = row_segs[seg_of_h[a + wlen - 1]][1]
        ks = [k for k in range(Kh) if k * P < h_hi and (k + 1) * P > h_lo]
        windows.append((a, wlen, ks))

    return dict(
        S_h=S_h, S_w=S_w, Kh=Kh,
        wperm=wperm, slot_bases=slot_bases, cls_x0=cls_x0,
        mm_groups=mm_groups, exp_pieces=exp_pieces, n_chunks=n_chunks,
        chunk_out_end=chunk_out_end,
        seg_of_h=seg_of_h, windows=windows,
    )


def _build_program(row_segs, col_segs, plan):
    import concourse.mybir as mybir
    import concourse.tile as tile
    from concourse import bacc

    fp16 = mybir.dt.float16
    fp32 = mybir.dt.float32
    COPY = mybir.ActivationFunctionType.Copy

    Kh = plan["Kh"]
    S_w = plan["S_w"]
    CPW = S_w * CS  # pooled row free size

    nc = bacc.Bacc()
    x = nc.dram_tensor("x", [H, FW], fp16, kind="ExternalInput")
    fM = nc.dram_tensor("fM", [H, H], fp16, kind="ExternalInput")
    y = nc.dram_tensor("y", [H, FW], fp16, kind="ExternalOutput")

    with tile.TileContext(nc) as tc:
        with (
            tc.tile_pool(name="consts", bufs=1) as consts,
            tc.tile_pool(name="ot", bufs=2) as otpool,
            tc.tile_pool(name="ps2", bufs=3, space="PSUM") as ps2pool,
            tc.tile_pool(name="warm", bufs=1, space="PSUM") as warmpool,
        ):
            # stationary fused pool+expand matrix, one tile per h-chunk
            fM_sb = []
            for k in range(Kh):
                t = consts.tile([P, H], fp16, name=f"fM{k}")
                nc.sync.dma_start(t[:], fM[k * P:(k + 1) * P, :])
                fM_sb.append(t)

            # x resident in SBUF, one tile per h-chunk, loaded in slices
            # on the Activation HWDGE ring (stores keep the SP ring)
            x_sb = []
            for k in range(Kh):
                t = consts.tile([P, FW], fp16, name=f"x{k}")
                step = FW // LOAD_SPLIT
                for s0 in range(0, FW, step):
                    nc.scalar.dma_start(
                        t[:, s0:s0 + step],
                        x[k * P:(k + 1) * P, s0:s0 + step],
                    )
                x_sb.append(t)

            # PE pre-touch of every stationary tile: later matmuls then
            # reach the operand without a DMA wait (keeps the LDWEIGHTS
            # sync-wait count within the ISA limit).
            ps_warm = warmpool.tile([1, PSW], fp32, name="ps_warm")
            for t in fM_sb:
                nc.tensor.matmul(
                    ps_warm[:1, :1], t[:, :1], t[:, :1],
                    start=True, stop=True,
                )

            # --------- fused pool+expand per output window ---------------
            exp_rr = 0
            for (a, wlen, ks) in plan["windows"]:
                ot = otpool.tile([P, FW], fp16, tag="ot", name=f"ot{a}")
                store_lo = 0
                for ci in range(plan["n_chunks"]):
                    c0 = ci * CHUNK2
                    ps2 = ps2pool.tile([P, CHUNK2], fp32, tag="ps2",
                                       name=f"ps2_{a}_{ci}")
                    for hi in (2 * ci, 2 * ci + 1):
                        for (ps_off, width, x_offs) in plan["mm_groups"][hi]:
                            tot = len(ks) * len(x_offs)
                            idx = 0
                            for k in ks:
                                for x_off in x_offs:
                                    nc.tensor.matmul(
                                        ps2[:wlen,
                                            ps_off - c0:
                                            ps_off - c0 + width],
                                        fM_sb[k][:, a:a + wlen],
                                        x_sb[k][:, x_off:x_off + width],
                                        start=(idx == 0),
                                        stop=(idx == tot - 1),
                                    )
                                    idx += 1
                    for (L, n, slot0, lw0) in plan["exp_pieces"][ci]:
                        src = ps2[:wlen,
                                  slot0 * CS - c0:(slot0 + n) * CS - c0]
                        src = src.rearrange("p (j c) -> p j c",
                                            j=n, c=CS)
                        src = src.unsqueeze(2)
                        src = src.broadcast_to([wlen, n, L, CS])
                        dst = ot[:wlen, lw0 * CS:(lw0 + n * L) * CS]
                        dst = dst.rearrange("p (j l c) -> p j l c",
                                            j=n, l=L, c=CS)
                        if exp_rr % 2 == 0:
                            nc.vector.tensor_scalar_mul(dst, src,
                                                        1.0 / L)
                        else:
                            nc.scalar.activation(dst, src, COPY,
                                                 scale=1.0 / L)
                        exp_rr += 1
                half = FW // 2
                nc.sync.dma_start(y[a:a + wlen, :half],
                                  ot[:wlen, :half])
                nc.sync.dma_start(y[a:a + wlen, half:],
                                  ot[:wlen, half:])

    nc.compile()
    nc.finalize()
    return nc


def _prep_host(input, h_mask, v_mask):
    """Returns (nc, in_maps, plan) ready for execution."""
    row_segs = _segments(h_mask)
    col_segs = _segments(v_mask)
    plan = _plan(row_segs, col_segs)
    nc = _build_program(row_segs, col_segs, plan)
    in_maps = _make_in_maps(input, row_segs, plan)
    return nc, in_maps, plan


def _make_in_maps(input, row_segs, plan):
    # fused pool+expand matrix
    seg_of_h = plan["seg_of_h"]
    seg_len = np.empty(H, dtype=np.float64)
    for (va, vb) in row_segs:
        seg_len[va:vb] = vb - va
    fM = np.where(
        seg_of_h[:, None] == seg_of_h[None, :],
        (1.0 / seg_len)[:, None],
        0.0,
    ).astype(np.float16)

    # device x layout: class blocks in slot order, each [L, n, C] l-major
    xp16 = np.asarray(input)[0].astype(np.float16)  # [H, W, C]
    parts = []
    pos = 0
    for (L, n, slot0) in plan["slot_bases"]:
        cols = plan["wperm"][pos:pos + n * L]
        pos += n * L
        blk = xp16[:, cols, :].reshape(H, n, L, C)
        parts.append(np.ascontiguousarray(blk.transpose(0, 2, 1, 3)))

    in_maps = []
    for k in range(NCORES):
        xc = np.concatenate(
            [p[:, :, :, k * CS:(k + 1) * CS].reshape(H, -1) for p in parts],
            axis=1,
        )
        in_maps.append({"x": np.ascontiguousarray(xc), "fM": fM})
    return in_maps


# stash for test.py introspection
LAST_RESULT = {}
_EXEC_CACHE = {}


def _make_executable(nc):
    """Build a reusable sharded jit callable for this program.

    Mirrors bass2jax.run_bass_via_pjrt's multi-core branch but keeps the
    jitted function so repeated calls skip retrace/recompile (and so the
    test harness can time steady-state executions).
    """
    import jax
    import concourse.mybir as mybir
    from concourse import bass2jax
    from jax.sharding import Mesh, PartitionSpec
    from jax.experimental.shard_map import shard_map

    bass2jax.install_neuronx_cc_hook()

    partition_name = (
        nc.partition_id_tensor.name if nc.partition_id_tensor else None
    )
    in_names, out_names, out_shapes, out_dtypes = [], [], [], []
    for alloc in nc.m.functions[0].allocations:
        if not isinstance(alloc, mybir.MemoryLocationSet):
            continue
        name = alloc.memorylocations[0].name
        if alloc.kind == "ExternalInput":
            if name != partition_name:
                in_names.append(name)
        elif alloc.kind == "ExternalOutput":
            out_names.append(name)
            out_shapes.append(tuple(alloc.tensor_shape))
            out_dtypes.append(mybir.dt.np(alloc.dtype))
    out_avals = tuple(
        jax.core.ShapedArray(s, d) for s, d in zip(out_shapes, out_dtypes)
    )
    n_params = len(in_names)
    n_outs = len(out_names)
    all_names = in_names + out_names
    if partition_name is not None:
        all_names = all_names + [partition_name]

    def _body(*args):
        operands = list(args)
        if partition_name is not None:
            operands.append(bass2jax.partition_id_tensor())
        outs = bass2jax._bass_exec_p.bind(
            *operands,
            out_avals=out_avals,
            in_names=tuple(all_names),
            out_names=tuple(out_names),
            lowering_input_output_aliases=(),
            sim_require_finite=True,
            sim_require_nnan=True,
            nc=nc,
        )
        return tuple(outs)

    devices = jax.devices()[:NCORES]
    mesh = Mesh(np.asarray(devices), ("core",))
    donate = tuple(range(n_params, n_params + n_outs))
    sharded = jax.jit(
        shard_map(
            _body,
            mesh=mesh,
            in_specs=(PartitionSpec("core"),) * (n_params + n_outs),
            out_specs=(PartitionSpec("core"),) * n_outs,
            check_rep=False,
        ),
        donate_argnums=donate,
        keep_unused=True,
    )

    def run(in_maps):
        concat_in = [
            np.concatenate([m[name] for m in in_maps], axis=0)
            for name in in_names
        ]
        concat_zeros = [
            np.zeros((NCORES * s[0], *s[1:]), d)
            for s, d in zip(out_shapes, out_dtypes)
        ]
        out_arrs = sharded(*concat_in, *concat_zeros)
        return [
            {
                name: np.asarray(out_arrs[i]).reshape(
                    NCORES, *out_shapes[i]
                )[c]
                for i, name in enumerate(out_names)
            }
            for c in range(NCORES)
        ]

    return run


def _get_run(input, h_mask, v_mask):
    key = (np.asarray(h_mask).tobytes(), np.asarray(v_mask).tobytes())
    if key not in _EXEC_CACHE:
        nc, in_maps, plan = _prep_host(
            np.asarray(input), np.asarray(h_mask), np.asarray(v_mask)
        )
        LAST_RESULT["nc"] = nc
        _EXEC_CACHE[key] = (_make_executable(nc), plan)
    else:
        row_segs = _segments(h_mask)
        plan = _EXEC_CACHE[key][1]
        in_maps = _make_in_maps(np.asarray(input), row_segs, plan)
    return _EXEC_CACHE[key][0], in_maps


def kernel(input, h_mask, v_mask):
    run, in_maps = _get_run(input, h_mask, v_mask)
    results = run(in_maps)
    LAST_RESULT["results"] = results

    key = (np.asarray(h_mask).tobytes(), np.asarray(v_mask).tobytes())
    plan = _EXEC_CACHE[key][1]
    # un-permute the cols (class-sorted) and upcast
    winv = np.empty(W, dtype=np.int64)
    winv[plan["wperm"]] = np.arange(W)

    out = np.empty((H, W, C), dtype=np.float32)
    for k in range(NCORES):
        yk = results[k]["y"].reshape(H, W, CS)
        out[:, :, k * CS:(k + 1) * CS] = yk[:, winv]
    return out[None]
